# revision 1
# baseline (speedup 1.0000x reference)
"""CDR-aligned conditioner kernel for Trainium2 (8 NeuronCores).

Strategy
--------
The reference projects every text token through a 2-layer MLP
(3584 -> 768 -> SiLU -> 384) and then, per (chain_type, cdr_type) pair,
copies the k-th masked text row to the k-th masked protein position.
Only protein positions that receive a row are nonzero in the output, and
with these masks that's only ~300 of 2048 rows per batch element.  So:

1. (host) compute the aligned (batch, text_src, protein_dst) triples
   with cheap integer ops — exactly the reference's cumsum/rank logic;
2. (host) gather just those text rows;
3. (device, 8 cores data-parallel over rows) dense MLP on the gathered
   rows:  Y^T = (scale*W2) @ silu(W1 @ X^T + b1) + scale*b2;
4. (host) scatter the projected rows into the zero-initialized output.

Device kernel notes:
- everything is k-major in SBUF; GEMM1 runs k-outer/h-inner so each
  557KB k-tile (x + w1 slices) is consumed as soon as its DMA lands,
  with 6 PSUM banks accumulating the 6 h-tiles concurrently.
- x/w1 DMAs are issued first (k-interleaved) so the PE starts as early
  as possible; W2 arrives while GEMM1 runs.
- biases enter as augmented contraction tiles (ones-row in the moving
  operand, bias-row in the stationary operand) and cost nothing when
  they are all-zero (the common case here).
- matmul dtype float32r: full-rate PE (1 cycle/row vs 4 for fp32) with
  reduced-precision multiplies; measured ~2e-4 max relative error vs
  the fp32 reference on this problem (fp32 mode available via
  CDR_MM_DTYPE=f32 at ~2.3x the time, ~8e-7 rel err).

Measured on trn2 (8 cores, NTFF profile): ~65 us HW exec per launch;
the per-core ingest (11MB W1^T + 4.5MB X^T + 1.2MB W2^T) saturates the
~358 GB/s per-core HBM ceiling for ~45 us of that, with the PE stream
(186 fused LDWEIGHTS+MATMUL pairs at ~267 ns) hidden underneath.
"""

import os
import sys

sys.path.insert(0, "/opt/trn_rl_repo")

import ml_dtypes
import numpy as np

import concourse.bass as bass
import concourse.mybir as mybir
from concourse import bacc
from concourse.bass_utils import run_bass_kernel_spmd
from concourse.tile import TileContext

# Problem constants (hardcoded per contract)
B, L_TEXT, N_TOKEN = 8, 2048, 2048
C_TEXT, C_OUT = 3584, 384
C_HID = C_OUT * 2
CHAIN_TYPES = (1, 2)
CDR_TYPES = (2, 4, 6)
N_CORES = 8

KT = C_TEXT // 128   # 28 k-tiles (contraction of GEMM1)
HT = C_HID // 128    # 6 h-tiles
OT = C_OUT // 128    # 3 out-tiles

F32 = mybir.dt.float32
AF = mybir.ActivationFunctionType

# matmul element type: float32r (default) or float32 (exact, ~2x slower)
_MM_DT_NAME = os.environ.get("CDR_MM_DTYPE", "f32r")

_kernel_cache = {}

# test harness hooks: set _TRACE=True to profile; exec times land in
# _last_exec_ns (one entry per device launch).
_TRACE = False
_last_exec_ns = []
_last_results = []


def _build_mlp_kernel(cap: int, has_b1: bool, has_b2: bool):
    """Dense MLP on `cap` gathered rows."""
    mm_dt = {"f32": F32, "f32r": mybir.dt.float32r,
             "bf16": mybir.dt.bfloat16, "f16": mybir.dt.float16}[_MM_DT_NAME]
    kt_x = KT + (1 if has_b1 else 0)   # augmented contraction tiles

    nc = bacc.Bacc("TRN2", target_bir_lowering=False, debug=False,
                   num_devices=N_CORES)
    xT = nc.declare_dram_parameter("xT", [kt_x * 128, cap], mm_dt, isOutput=False)
    w1T = nc.declare_dram_parameter("w1T", [kt_x * 128, C_HID], mm_dt, isOutput=False)
    w2T = nc.declare_dram_parameter("w2T", [C_HID, C_OUT], mm_dt, isOutput=False)
    if has_b2:
        b2 = nc.declare_dram_parameter("b2", [1, C_OUT], F32, isOutput=False)
    out = nc.declare_dram_parameter("out", [C_OUT, cap], F32, isOutput=True)

    xT_r = xT.rearrange("(n p) c -> n p c", p=128)
    w1T_r = w1T.rearrange("(n p) h -> n p h", p=128)
    w2T_r = w2T.rearrange("(n p) c -> n p c", p=128)

    with TileContext(nc) as tc:
        with (
            tc.tile_pool(name="persist", bufs=1) as pp,
            tc.tile_pool(name="psum", bufs=1, space="PSUM") as psum_pool,
        ):
            w1_sb = [pp.tile([128, C_HID], mm_dt, name=f"w1_sb{k}", tag=f"w1{k}")
                     for k in range(kt_x)]
            x_sb = [pp.tile([128, cap], mm_dt, name=f"x_sb{k}", tag=f"x{k}")
                    for k in range(kt_x)]
            w2_sb = [pp.tile([128, C_OUT], mm_dt, name=f"w2_sb{h}", tag=f"w2{h}")
                     for h in range(HT)]
            h_sb = pp.tile([128, HT * cap], mm_dt, name="h_sb", tag="h")
            y_sb = pp.tile([128, OT * cap], F32, name="y_sb", tag="y")
            if has_b2:
                b2_sb = pp.tile([1, C_OUT], F32, name="b2_sb", tag="b2")
                ones_sb = pp.tile([1, cap], F32, name="ones_sb", tag="ones")

            # x/w1 first, k-interleaved: the GEMM1 k-loop consumes tiles
            # in exactly this order.
            # w2 tiles interleaved one-per-k over k=16..21: the PE's
            # pipeline lead absorbs each small displacement, unlike a
            # single 6-tile block which drains it (measured 3.3us stall)
            w2_at = {16 + i: i for i in range(HT)} if kt_x > 21 else {}
            for k in range(kt_x):
                nc.sync.dma_start(out=x_sb[k][:], in_=xT_r[k])
                nc.sync.dma_start(out=w1_sb[k][:], in_=w1T_r[k])
                if k in w2_at:
                    h = w2_at[k]
                    nc.sync.dma_start(out=w2_sb[h][:], in_=w2T_r[h])
            if not w2_at:
                for h in range(HT):
                    nc.sync.dma_start(out=w2_sb[h][:], in_=w2T_r[h])
            if has_b2:
                nc.sync.dma_start(out=b2_sb[:], in_=b2[:])
                nc.gpsimd.memset(ones_sb[:], 1.0)

            # GEMM1: k-outer / h-inner; 6 PSUM banks accumulate concurrently.
            ps1 = [psum_pool.tile([128, cap], F32, name=f"ps1_{h}", tag=f"ps1_{h}")
                   for h in range(HT)]
            for k in range(kt_x):
                for h in range(HT):
                    nc.tensor.matmul(
                        ps1[h][:],
                        lhsT=w1_sb[k][:, h * 128:(h + 1) * 128],
                        rhs=x_sb[k][:],
                        start=(k == 0),
                        stop=(k == kt_x - 1),
                    )
            for h in range(HT):
                nc.scalar.activation(h_sb[:, h * cap:(h + 1) * cap], ps1[h][:],
                                     AF.Silu)

            # GEMM2 (+ optional bias via K=1 ones-row matmul), h-outer so
            # each h-chunk is consumed as soon as its silu completes
            ps2 = [psum_pool.tile([128, cap], F32, name=f"ps2_{c}", tag=f"ps1_{c}")
                   for c in range(OT)]
            for h in range(HT):
                for c in range(OT):
                    nc.tensor.matmul(
                        ps2[c][:],
                        lhsT=w2_sb[h][:, c * 128:(c + 1) * 128],
                        rhs=h_sb[:, h * cap:(h + 1) * cap],
                        start=(h == 0),
                        stop=(h == HT - 1) and not has_b2,
                    )
            for c in range(OT):
                if has_b2:
                    nc.tensor.matmul(
                        ps2[c][:],
                        lhsT=b2_sb[:, c * 128:(c + 1) * 128],
                        rhs=ones_sb[:],
                        start=False,
                        stop=True,
                    )
                nc.scalar.activation(y_sb[:, c * cap:(c + 1) * cap], ps2[c][:],
                                     AF.Copy)
                nc.sync.dma_start(out=out[c * 128:(c + 1) * 128, :],
                                  in_=y_sb[:, c * cap:(c + 1) * cap])
    nc.compile()
    return nc


def _get_kernel(cap: int, has_b1: bool, has_b2: bool):
    key = (cap, has_b1, has_b2, _MM_DT_NAME)
    if key not in _kernel_cache:
        _kernel_cache[key] = _build_mlp_kernel(cap, has_b1, has_b2)
    return _kernel_cache[key]


def _alignment_indices(text_mask, chain_type_ids, cdr_region_type_ids,
                       boltz_chain_type, boltz_region_type):
    """All (b, text_src, protein_dst) triples, reference semantics."""
    tm = text_mask.astype(bool)
    bs, srcs, dsts = [], [], []
    for b in range(B):
        for ct in CHAIN_TYPES:
            for rt in CDR_TYPES:
                tmask = (chain_type_ids[b] == ct) & (cdr_region_type_ids[b] == rt) & tm[b]
                pmask = (boltz_chain_type[b] == ct) & (boltz_region_type[b] == rt)
                ti = np.nonzero(tmask)[0]
                pi = np.nonzero(pmask)[0]
                k = min(ti.shape[0], pi.shape[0])
                if k:
                    bs.append(np.full(k, b, np.int64))
                    srcs.append(ti[:k])
                    dsts.append(pi[:k])
    if not bs:
        z = np.zeros(0, np.int64)
        return z, z, z
    return np.concatenate(bs), np.concatenate(srcs), np.concatenate(dsts)


def kernel(text_conditioning, text_mask, chain_type_ids, cdr_region_type_ids,
           boltz_chain_type, boltz_region_type, W1, b1, W2, b2, scale):
    text_conditioning = np.asarray(text_conditioning, np.float32)
    W1 = np.asarray(W1, np.float32)
    b1v = np.asarray(b1, np.float32).reshape(-1)
    W2 = np.asarray(W2, np.float32)
    b2v = np.asarray(b2, np.float32).reshape(-1)
    scale_v = np.float32(np.asarray(scale).reshape(-1)[0])

    all_b, all_src, all_dst = _alignment_indices(
        np.asarray(text_mask), np.asarray(chain_type_ids),
        np.asarray(cdr_region_type_ids), np.asarray(boltz_chain_type),
        np.asarray(boltz_region_type))

    result = np.zeros((B, N_TOKEN, C_OUT), np.float32)
    nr = all_b.shape[0]
    if nr == 0:
        return result

    has_b1 = bool(b1v.any())
    b2s = b2v * scale_v
    has_b2 = bool(b2s.any())

    npdt = {"f32": np.float32, "f32r": np.float32,
            "bf16": ml_dtypes.bfloat16, "f16": np.float16}[_MM_DT_NAME]
    # scale folds into the second layer
    w1T = np.ascontiguousarray(W1.T)                    # [3584, 768]
    if has_b1:
        aug = np.zeros((128, C_HID), np.float32)
        aug[0] = b1v
        w1T = np.concatenate([w1T, aug], axis=0)        # [3712, 768]
    w1T = w1T.astype(npdt)
    w2T = np.ascontiguousarray((W2 * scale_v).T).astype(npdt)  # [768, 384]

    x_rows = text_conditioning[all_b, all_src, :]       # [nr, 3584]

    per_launch_cap = 512
    launch_rows = N_CORES * per_launch_cap
    y_rows = np.empty((nr, C_OUT), np.float32)
    kt_x = KT + (1 if has_b1 else 0)

    for lo in range(0, nr, launch_rows):
        hi = min(nr, lo + launch_rows)
        n = hi - lo
        per_core = -(-n // N_CORES)
        cap = min(per_launch_cap, max(256, -(-per_core // 8) * 8))
        nc = _get_kernel(cap, has_b1, has_b2)
        in_maps = []
        bounds = []
        for c in range(N_CORES):
            a = lo + c * cap
            z = min(hi, a + cap)
            a = min(a, z)
            bounds.append((a, z))
            xT = np.zeros((kt_x * 128, cap), npdt)
            if z > a:
                xT[:C_TEXT, :z - a] = x_rows[a:z].T
                if has_b1:
                    xT[C_TEXT, :z - a] = 1.0
            m = {"xT": xT, "w1T": w1T, "w2T": w2T}
            if has_b2:
                m["b2"] = b2s.reshape(1, -1)
            in_maps.append(m)
        res = run_bass_kernel_spmd(nc, in_maps, list(range(N_CORES)),
                                   trace=_TRACE)
        if _TRACE:
            _last_exec_ns.append(res.exec_time_ns)
            _last_results.append(res)
        for c, (a, z) in enumerate(bounds):
            if z > a:
                y_rows[a:z] = res.results[c]["out"][:, :z - a].T

    result[all_b, all_dst, :] = y_rows
    return result



# revision 3
# speedup vs baseline: 1.4807x; 1.4807x over previous
"""CDR-aligned conditioner kernel for Trainium2 (8 NeuronCores).

Strategy
--------
The reference projects every text token through a 2-layer MLP
(3584 -> 768 -> SiLU -> 384) and then, per (chain_type, cdr_type) pair,
copies the k-th masked text row to the k-th masked protein position.
Only protein positions that receive a row are nonzero in the output
(~2460 of 16384 rows), so:

1. (host) compute the aligned (batch, text_src, protein_dst) triples
   with cheap integer ops — exactly the reference's cumsum/rank logic;
2. (host) gather just those text rows;
3. (device, 8 cores data-parallel over rows) dense MLP on the gathered
   rows:  Y^T = (scale*W2) @ silu(W1 @ X^T + b1) + scale*b2;
4. (host) scatter the projected rows into the zero-initialized output.

Device kernel notes (v2):
- fp16 operands: same 1 cycle/row PE rate as f32r but half the HBM
  traffic (~8MB/core), and ~4e-4 rel err vs the 2e-2 gate.
- All inputs prepacked on host to partition-major [128, ktiles*width]
  layouts so every DMA descriptor moves a multi-KB contiguous line
  (the v1 kernel's 624B lines ran DMA at ~184GB/s of the 360 peak).
- DMA issued in k-chunks (1,1,2,4,4,...) so the first matmul starts
  ~2.5us in and the PE then streams without starving (per 4-k-tile
  chunk: DMA ~3.1us vs PE ~3.7us).
- Tail: the last k-chunk of GEMM1 runs h-major so each silu(h) and its
  GEMM2 matmuls overlap the remaining GEMM1 work instead of forming a
  serial epilogue; output copies go on the vector engine to stay off
  the silu-busy scalar engine.
"""

import os
import sys

sys.path.insert(0, "/opt/trn_rl_repo")

import numpy as np

import concourse.bass as bass
import concourse.mybir as mybir
from concourse import bacc
from concourse.bass_utils import run_bass_kernel_spmd
from concourse.tile import TileContext

# Problem constants (hardcoded per contract)
B, L_TEXT, N_TOKEN = 8, 2048, 2048
C_TEXT, C_OUT = 3584, 384
C_HID = C_OUT * 2
CHAIN_TYPES = (1, 2)
CDR_TYPES = (2, 4, 6)
N_CORES = 8

KT = C_TEXT // 128   # 28 k-tiles (contraction of GEMM1)
HT = C_HID // 128    # 6 h-tiles
OT = C_OUT // 128    # 3 out-tiles

F32 = mybir.dt.float32
F16 = mybir.dt.float16
AF = mybir.ActivationFunctionType

_kernel_cache = {}

# test harness hooks: set _TRACE=True to profile; exec times land in
# _last_exec_ns (one entry per device launch).
_TRACE = False
_last_exec_ns = []
_last_results = []


def _chunk_sizes(kt_x: int) -> list:
    """k-tile DMA chunking: small leading chunks for a fast PE start,
    then 4-tile chunks that keep DMA slightly ahead of the PE."""
    sizes = [1, 1, 2]
    left = kt_x - sum(sizes)
    while left > 0:
        take = min(4, left)
        sizes.append(take)
        left -= take
    return sizes


def _build_mlp_kernel(cap: int, has_b1: bool, has_b2: bool):
    """Dense MLP on `cap` gathered rows per core, fp16 operands."""
    kt_x = KT + (1 if has_b1 else 0)   # augmented contraction tiles
    chunks = _chunk_sizes(kt_x)
    nch = len(chunks)

    nc = bacc.Bacc("TRN2", target_bir_lowering=False, debug=False,
                   num_devices=N_CORES)
    # partition-major packed layouts: [128, k*width]; column k*width+j of
    # partition p holds element (128k+p, j) of the k-major operand.
    w1p = nc.declare_dram_parameter("w1p", [128, kt_x * C_HID], F16,
                                    isOutput=False)
    xp = nc.declare_dram_parameter("xp", [128, kt_x * cap], F16,
                                   isOutput=False)
    w2p = nc.declare_dram_parameter("w2p", [128, HT * C_OUT], F16,
                                    isOutput=False)
    if has_b2:
        b2 = nc.declare_dram_parameter("b2", [1, C_OUT], F32, isOutput=False)
    out = nc.declare_dram_parameter("out", [128, OT * cap], F16,
                                    isOutput=True)

    with TileContext(nc) as tc:
        with (
            tc.tile_pool(name="persist", bufs=1) as pp,
            tc.tile_pool(name="psum", bufs=1, space="PSUM") as pq,
        ):
            w1_sb = [pp.tile([128, n * C_HID], F16, name=f"w1c{i}",
                             tag=f"w1c{i}") for i, n in enumerate(chunks)]
            x_sb = [pp.tile([128, n * cap], F16, name=f"xc{i}",
                            tag=f"xc{i}") for i, n in enumerate(chunks)]
            w2_sb = pp.tile([128, HT * C_OUT], F16, name="w2", tag="w2")
            h_sb = pp.tile([128, HT * cap], F16, name="h", tag="h")
            y_sb = pp.tile([128, OT * cap], F16, name="y", tag="y")
            if has_b2:
                b2_sb = pp.tile([1, C_OUT], F32, name="b2", tag="b2")
                ones_sb = pp.tile([1, cap], F32, name="ones", tag="ones")

            # DMA feed: per chunk, w1 then x; w2 mid-stream where the
            # queue has slack.
            cw = cx = 0
            w2_after = max(0, nch - 4)
            for i, n in enumerate(chunks):
                nc.sync.dma_start(out=w1_sb[i][:],
                                  in_=w1p[:, cw:cw + n * C_HID])
                nc.sync.dma_start(out=x_sb[i][:],
                                  in_=xp[:, cx:cx + n * cap])
                cw += n * C_HID
                cx += n * cap
                if i == w2_after:
                    nc.sync.dma_start(out=w2_sb[:], in_=w2p[:])
                    if has_b2:
                        nc.sync.dma_start(out=b2_sb[:], in_=b2[:])
                        nc.gpsimd.memset(ones_sb[:], 1.0)

            ps1 = [pq.tile([128, cap], F32, name=f"ps1_{h}", tag=f"ps1_{h}")
                   for h in range(HT)]
            # ps2_c shares ps1_c's slot (released after silu(c) reads it),
            # keeping the pool within the 8 PSUM banks.
            ps2 = [pq.tile([128, cap], F32, name=f"ps2_{c}", tag=f"ps1_{c}")
                   for c in range(OT)]

            def g1mm(ci, j, h, start, stop):
                nc.tensor.matmul(
                    ps1[h][:],
                    lhsT=w1_sb[ci][:, j * C_HID + h * 128:
                                   j * C_HID + (h + 1) * 128],
                    rhs=x_sb[ci][:, j * cap:(j + 1) * cap],
                    start=start, stop=stop,
                )

            def g2mm(h):
                for c in range(OT):
                    nc.tensor.matmul(
                        ps2[c][:],
                        lhsT=w2_sb[:, h * C_OUT + c * 128:
                                   h * C_OUT + (c + 1) * 128],
                        rhs=h_sb[:, h * cap:(h + 1) * cap],
                        start=(h == 0),
                        stop=(h == HT - 1) and not has_b2,
                    )

            # Phase A: k-outer / h-inner over all but the last chunk.
            kglob = 0
            for i, n in enumerate(chunks[:-1]):
                for j in range(n):
                    for h in range(HT):
                        g1mm(i, j, h, start=(kglob == 0), stop=False)
                    kglob += 1

            # Phase B: last chunk h-major; silu(h) and GEMM2(h) overlap
            # the remaining GEMM1 matmuls.
            li, ln = nch - 1, chunks[-1]
            for h in range(HT):
                for j in range(ln):
                    g1mm(li, j, h, start=False, stop=(j == ln - 1))
                nc.scalar.activation(h_sb[:, h * cap:(h + 1) * cap],
                                     ps1[h][:], AF.Silu)
                if h >= 1:
                    g2mm(h - 1)
            g2mm(HT - 1)

            for c in range(OT):
                if has_b2:
                    nc.tensor.matmul(
                        ps2[c][:],
                        lhsT=b2_sb[:, c * 128:(c + 1) * 128],
                        rhs=ones_sb[:],
                        start=False, stop=True,
                    )
                nc.vector.tensor_copy(y_sb[:, c * cap:(c + 1) * cap],
                                      ps2[c][:])
                nc.sync.dma_start(out=out[:, c * cap:(c + 1) * cap],
                                  in_=y_sb[:, c * cap:(c + 1) * cap])
    nc.compile()
    return nc


def _get_kernel(cap: int, has_b1: bool, has_b2: bool):
    key = (cap, has_b1, has_b2)
    if key not in _kernel_cache:
        _kernel_cache[key] = _build_mlp_kernel(cap, has_b1, has_b2)
    return _kernel_cache[key]


def _alignment_indices(text_mask, chain_type_ids, cdr_region_type_ids,
                       boltz_chain_type, boltz_region_type):
    """All (b, text_src, protein_dst) triples, reference semantics."""
    tm = text_mask.astype(bool)
    bs, srcs, dsts = [], [], []
    for b in range(B):
        for ct in CHAIN_TYPES:
            for rt in CDR_TYPES:
                tmask = (chain_type_ids[b] == ct) & (cdr_region_type_ids[b] == rt) & tm[b]
                pmask = (boltz_chain_type[b] == ct) & (boltz_region_type[b] == rt)
                ti = np.nonzero(tmask)[0]
                pi = np.nonzero(pmask)[0]
                k = min(ti.shape[0], pi.shape[0])
                if k:
                    bs.append(np.full(k, b, np.int64))
                    srcs.append(ti[:k])
                    dsts.append(pi[:k])
    if not bs:
        z = np.zeros(0, np.int64)
        return z, z, z
    return np.concatenate(bs), np.concatenate(srcs), np.concatenate(dsts)


def _pack_kmajor(arr_t, kt, width, dtype=np.float16):
    """[kt*128, width] -> [128, kt*width] partition-major packing."""
    a = np.asarray(arr_t, dtype)
    a = a.reshape(kt, 128, width).transpose(1, 0, 2).reshape(128, kt * width)
    return np.ascontiguousarray(a)


def kernel(text_conditioning, text_mask, chain_type_ids, cdr_region_type_ids,
           boltz_chain_type, boltz_region_type, W1, b1, W2, b2, scale):
    text_conditioning = np.asarray(text_conditioning, np.float32)
    W1 = np.asarray(W1, np.float32)
    b1v = np.asarray(b1, np.float32).reshape(-1)
    W2 = np.asarray(W2, np.float32)
    b2v = np.asarray(b2, np.float32).reshape(-1)
    scale_v = np.float32(np.asarray(scale).reshape(-1)[0])

    all_b, all_src, all_dst = _alignment_indices(
        np.asarray(text_mask), np.asarray(chain_type_ids),
        np.asarray(cdr_region_type_ids), np.asarray(boltz_chain_type),
        np.asarray(boltz_region_type))

    result = np.zeros((B, N_TOKEN, C_OUT), np.float32)
    nr = all_b.shape[0]
    if nr == 0:
        return result

    has_b1 = bool(b1v.any())
    b2s = b2v * scale_v
    has_b2 = bool(b2s.any())
    kt_x = KT + (1 if has_b1 else 0)

    # scale folds into the second layer
    w1T = np.ascontiguousarray(W1.T).astype(np.float16)   # [3584, 768]
    if has_b1:
        aug = np.zeros((128, C_HID), np.float16)
        aug[0] = b1v.astype(np.float16)
        w1T = np.concatenate([w1T, aug], axis=0)          # [3712, 768]
    w1_packed = _pack_kmajor(w1T, kt_x, C_HID)            # [128, kt_x*768]
    w2T = np.ascontiguousarray((W2 * scale_v).T).astype(np.float16)
    w2_packed = _pack_kmajor(w2T, HT, C_OUT)              # [128, 6*384]

    x_rows = text_conditioning[all_b, all_src, :].astype(np.float16)

    per_launch_cap = 512
    launch_rows = N_CORES * per_launch_cap
    y_rows = np.empty((nr, C_OUT), np.float32)

    for lo in range(0, nr, launch_rows):
        hi = min(nr, lo + launch_rows)
        per_core = -(-(hi - lo) // N_CORES)
        cap = min(per_launch_cap, max(64, -(-per_core // 8) * 8))
        nc = _get_kernel(cap, has_b1, has_b2)
        in_maps = []
        bounds = []
        for c in range(N_CORES):
            a = lo + c * cap
            z = min(hi, a + cap)
            a = min(a, z)
            bounds.append((a, z))
            xt = np.zeros((128, kt_x, cap), np.float16)
            if z > a:
                xt[:, :KT, :z - a] = (
                    x_rows[a:z].T.reshape(KT, 128, z - a).transpose(1, 0, 2))
                if has_b1:
                    xt[0, KT, :z - a] = 1.0
            m = {"xp": np.ascontiguousarray(xt.reshape(128, kt_x * cap)),
                 "w1p": w1_packed, "w2p": w2_packed}
            if has_b2:
                m["b2"] = b2s.reshape(1, -1)
            in_maps.append(m)
        res = run_bass_kernel_spmd(nc, in_maps, list(range(N_CORES)),
                                   trace=_TRACE)
        if _TRACE:
            _last_exec_ns.append(res.exec_time_ns)
            _last_results.append(res)
        for c, (a, z) in enumerate(bounds):
            if z > a:
                o = res.results[c]["out"]                 # [128, OT*cap] f16
                y = o.reshape(128, OT, cap).transpose(1, 0, 2).reshape(
                    OT * 128, cap)
                y_rows[a:z] = y[:, :z - a].T.astype(np.float32)

    result[all_b, all_dst, :] = y_rows
    return result
